# revision 1
# baseline (speedup 1.0000x reference)
# MoE top-2 routing kernel for 8 Trainium2 NeuronCores (expert-parallel).
#
# Problem (hardcoded shapes): T=2048 tokens, D=2048 model dim, F=4096 ffn dim,
# E=8 experts, top-2 routing with renormalized softmax weights.
#
# Sharding: one expert per core. The host does only data placement: a cheap
# fp32 router pre-pass picks each token's top-2 experts (selection is
# numerically unambiguous: min 2nd-vs-3rd logit gap is ~7e-4 for these
# inputs, 100x above fp32 matmul noise), gathers each expert's tokens into a
# fixed-capacity transposed buffer xT_e [D, C], and zero-pads the tail.
# Zero-padded token columns are provably harmless: MLP(0) = 0, so any router
# weight the device computes for them multiplies zero.
#
# The device computes the whole module for its tokens: router logits (full
# fp32 matmul), top-2 softmax weights, gate/up matmuls (float32r), silu,
# down matmul (float32r), and the per-token weight scaling. Output is
# y_e [C, D]; the host scatter-adds rows back to [T, D] (each token appears
# on exactly its 2 routed cores).
#
# PE structure: fp32r matmuls are self-loading (a ~193ns LDWEIGHTS per
# matmul), so all MLP matmuls keep the *weights moving* with N=512 and the
# activations stationary — the weight load hides under each 512-column
# matmul. Gate/up produce g,u in [t, f]; h is PE-transposed to [f, t] tiles
# for the down matmul, which then produces y in natural [t, d] layout.
# Each f-chunk's transpose+down work is deferred by one f-chunk so the PE
# never stalls on the silu/mul/evict chain. Measured on HW: fp32r 559us
# (2.2e-4 scale-rel absmax err), bf16 490us (3.7e-3).

import os
import numpy as np
import ml_dtypes

_BF16NP = ml_dtypes.bfloat16

import concourse.bass as bass
import concourse.bacc as bacc
import concourse.mybir as mybir
import concourse.tile as tile
from concourse.masks import make_identity
from concourse import bass_utils

FP32 = mybir.dt.float32
FP32R = mybir.dt.float32r
BF16 = mybir.dt.bfloat16
# MLP matmul dtype: bf16 (1 cyc/col, ~4e-3 scale-rel err) vs fp32r
# (1.25 cyc/col, ~2e-4). Router always full fp32.
USE_BF16 = os.environ.get("MOE_BF16", "0") == "1"
# Experimental: accumulate down-projection partials into DRAM via DMA
# accum_op=add (frees the SBUF y accumulator for deeper weight prefetch).
ACC_DMA = os.environ.get("MOE_ACC_DMA", "0") == "1"
AX = mybir.AxisListType
ALU = mybir.AluOpType
ACTF = mybir.ActivationFunctionType

T, D, F, E = 2048, 2048, 4096, 8
NCORES = 8
ND = D // 128    # 16 d-tiles (contraction for gate/up)
NF = F // 128    # 32 f-tiles (contraction for down)
NFC = F // 512   # 8 moving f-chunks for gate/up
NDC = D // 512   # 4 moving d-chunks for down


def _chunks_for(C):
    """Split C token columns into PSUM-bank-sized chunks (<=512, mult of 64)."""
    nch = (C + 511) // 512
    out, rem, c0 = [], C, 0
    for i in range(nch):
        cn = -(-(rem // (nch - i)) // 64) * 64
        cn = min(cn, rem)
        out.append((c0, cn))
        c0 += cn
        rem -= cn
    return out


def build_program(C, use_bf16=USE_BF16):
    MDT = BF16 if use_bf16 else FP32R
    # fp32r tiles are 2x the bytes of bf16 — shrink pools to fit SBUF
    W_BUFS = 64 if use_bf16 else (58 if ACC_DMA else 43)
    HCH_BUFS = 12 if use_bf16 else 8
    HTC_BUFS = 2 if use_bf16 else 1
    NT = C // 128             # token tiles
    rchunks = _chunks_for(C)  # router-only chunking
    nc = bacc.Bacc(
        "TRN2",
        target_bir_lowering=False,
        debug=False,
        enable_asserts=False,
        num_devices=NCORES,
    )
    xT_d = nc.dram_tensor("xT", [D, C], FP32, kind="ExternalInput").ap()
    rw_d = nc.dram_tensor("rw", [D, E], FP32, kind="ExternalInput").ap()
    eoh_d = nc.dram_tensor("eoh", [1, E], FP32, kind="ExternalInput").ap()
    wg_d = nc.dram_tensor("wg", [D, F], MDT, kind="ExternalInput").ap()
    wu_d = nc.dram_tensor("wu", [D, F], MDT, kind="ExternalInput").ap()
    wd_d = nc.dram_tensor("wd", [F, D], MDT, kind="ExternalInput").ap()
    y_d = nc.dram_tensor("y", [C, D], FP32, kind="ExternalOutput").ap()

    with tile.TileContext(nc) as tc:
        with (
            tc.tile_pool(name="const", bufs=1) as const_pool,
            tc.tile_pool(name="x", bufs=1) as x_pool,
            tc.tile_pool(name="yacc", bufs=1) as yacc_pool,
            tc.tile_pool(name="htc", bufs=HTC_BUFS) as htc_pool,
            tc.tile_pool(name="hch", bufs=HCH_BUFS) as hch_pool,
            tc.tile_pool(name="w", bufs=W_BUFS) as w_pool,
            tc.tile_pool(name="tmp", bufs=4) as tmp_pool,
            tc.tile_pool(name="ps", bufs=8, space="PSUM") as ps_pool,
        ):
            # ---- constants / small inputs ----
            ident = const_pool.tile([128, 128], FP32, tag="ident", name="ident")
            make_identity(nc, ident[:])
            identm = const_pool.tile([128, 128], MDT, tag="identm", name="identm")
            if use_bf16:
                make_identity(nc, identm[:])
            else:
                # memset/iota can't write fp32r; round-copy the fp32 identity
                nc.vector.tensor_copy(identm[:], ident[:])
            rw_sb = const_pool.tile([128, ND * E], FP32, tag="rw", name="rw_sb")
            nc.sync.dma_start(
                rw_sb[:].rearrange("p (n e) -> p n e", e=E),
                rw_d.rearrange("(n p) e -> p n e", p=128),
            )
            eoh_sb = const_pool.tile([1, E], FP32, tag="eoh", name="eoh_sb")
            nc.sync.dma_start(eoh_sb[:], eoh_d[:])
            ones_sb = const_pool.tile([1, 128], FP32, tag="ones", name="ones")
            nc.vector.memset(ones_sb[:], 1.0)

            # ---- expert one-hot broadcast to [128, E] ----
            pe = ps_pool.tile([128, E], FP32, tag="ps", name="ps")
            nc.tensor.matmul(pe[:], ones_sb[:], eoh_sb[:], start=True, stop=True)
            eoh_b = const_pool.tile([128, E], FP32, tag="eohb", name="eohb")
            nc.scalar.copy(eoh_b[:], pe[:])

            # ---- one x pass: fp32 router logits + MDT residency ----
            xt = [x_pool.tile([128, C], MDT, tag=f"xt{d}", name=f"xt{d}")
                  for d in range(ND)]
            lT_sb = const_pool.tile([8, C], FP32, tag="lT", name="lT_sb")
            pls = [ps_pool.tile([8, cn], FP32, tag="ps", name="ps")
                   for (c0, cn) in rchunks]
            for d in range(ND):
                xf = tmp_pool.tile([128, C], FP32, tag="xf", name="xf", bufs=2)
                nc.sync.dma_start(xf[:], xT_d[d * 128:(d + 1) * 128, :])
                for pl, (c0, cn) in zip(pls, rchunks):
                    nc.tensor.matmul(
                        pl[:],
                        rw_sb[:, d * E:(d + 1) * E],
                        xf[:, c0:c0 + cn],
                        start=(d == 0),
                        stop=(d == ND - 1),
                    )
                nc.vector.tensor_copy(xt[d][:], xf[:])
            for pl, (c0, cn) in zip(pls, rchunks):
                nc.scalar.copy(lT_sb[:, c0:c0 + cn], pl[:])

            # ---- per-token top-2 softmax weight for this core's expert ----
            # wv[i] [128, 1] = weight of this expert for token tile i
            wv = []
            for i in range(NT):
                ptr = ps_pool.tile([128, E], FP32, tag="ps", name="ps")
                nc.tensor.transpose(ptr[:], lT_sb[:, i * 128:(i + 1) * 128], ident[:8, :8])
                lg = tmp_pool.tile([128, E], FP32, tag="lg", name="lg")
                nc.scalar.copy(lg[:], ptr[:])
                m1 = tmp_pool.tile([128, 1], FP32, tag="m1", name="m1")
                nc.vector.reduce_max(m1[:], lg[:], axis=AX.X)
                mask = tmp_pool.tile([128, E], FP32, tag="mask", name="mask")
                nc.vector.tensor_scalar(mask[:], lg[:], m1[:], None, op0=ALU.is_equal)
                masked = tmp_pool.tile([128, E], FP32, tag="masked", name="masked")
                nc.vector.scalar_tensor_tensor(
                    masked[:], mask[:], -1e30, lg[:], op0=ALU.mult, op1=ALU.add
                )
                m2 = tmp_pool.tile([128, 1], FP32, tag="m2", name="m2")
                nc.vector.reduce_max(m2[:], masked[:], axis=AX.X)
                le_t = tmp_pool.tile([128, E], FP32, tag="le_t", name="le_t")
                nc.vector.tensor_mul(le_t[:], lg[:], eoh_b[:])
                le = tmp_pool.tile([128, 1], FP32, tag="le", name="le")
                nc.vector.reduce_sum(le[:], le_t[:], axis=AX.X)
                nm1 = tmp_pool.tile([128, 1], FP32, tag="nm1", name="nm1")
                nc.vector.tensor_scalar_mul(nm1[:], m1[:], -1.0)
                e2 = tmp_pool.tile([128, 1], FP32, tag="e2", name="e2")
                nc.scalar.activation(e2[:], m2[:], ACTF.Exp, bias=nm1[:])
                den = tmp_pool.tile([128, 1], FP32, tag="den", name="den")
                nc.vector.tensor_scalar_add(den[:], e2[:], 1.0)
                rden = tmp_pool.tile([128, 1], FP32, tag="rden", name="rden")
                nc.vector.reciprocal(rden[:], den[:])
                ee = tmp_pool.tile([128, 1], FP32, tag="ee", name="ee")
                nc.scalar.activation(ee[:], le[:], ACTF.Exp, bias=nm1[:])
                wraw = tmp_pool.tile([128, 1], FP32, tag="wraw", name="wraw")
                nc.vector.tensor_mul(wraw[:], ee[:], rden[:])
                istop = tmp_pool.tile([128, 1], FP32, tag="istop", name="istop")
                nc.vector.tensor_tensor(istop[:], le[:], m2[:], op=ALU.is_ge)
                wvt = const_pool.tile([128, 1], FP32, tag=f"wv{i}", name=f"wv{i}")
                nc.vector.tensor_mul(wvt[:], wraw[:], istop[:])
                wv.append(wvt)

            # ---- fused MLP: per 512-wide f-chunk, gate/up -> h -> transpose
            # -> partial down, accumulating y in SBUF. Weights stream once. ----
            y_acc = ([] if ACC_DMA else
                     [yacc_pool.tile([128, D], FP32, tag=f"ya{t}", name=f"ya{t}")
                      for t in range(NT)])

            def emit_tr_down(fc, hch):
                """Transposes + partial down + y accumulation for f-chunk fc."""
                hTc = []
                for fs in range(4):
                    ht = htc_pool.tile([128, C], MDT, tag=f"htc{fs}", name=f"htc{fs}")
                    hTc.append(ht)
                for t in range(NT):
                    for fs in range(4):
                        ptr = ps_pool.tile([128, 128], MDT, tag="ps", name="ps")
                        nc.tensor.transpose(
                            ptr[:], hch[t][:, fs * 128:(fs + 1) * 128], identm[:]
                        )
                        nc.vector.tensor_copy(
                            hTc[fs][:, t * 128:(t + 1) * 128], ptr[:]
                        )
                wd_t = []
                for fs in range(4):
                    for dc in range(NDC):
                        wdt = w_pool.tile([128, 512], MDT, tag="w", name="wtile")
                        wsrc = wd_d[fc * 512 + fs * 128:fc * 512 + (fs + 1) * 128,
                                    dc * 512:(dc + 1) * 512]
                        nc.sync.dma_start(
                            wdt[:], wsrc if use_bf16 else wsrc.bitcast(FP32R)
                        )
                        wd_t.append(wdt)
                for t in range(NT):
                    for dc in range(NDC):
                        pp = ps_pool.tile([128, 512], FP32, tag="ps", name="ps")
                        for fs in range(4):
                            nc.tensor.matmul(
                                pp[:], hTc[fs][:, t * 128:(t + 1) * 128],
                                wd_t[fs * NDC + dc][:],
                                start=(fs == 0), stop=(fs == 3),
                            )
                        yslc = y_d[t * 128:(t + 1) * 128, dc * 512:(dc + 1) * 512]
                        if ACC_DMA:
                            yb = tmp_pool.tile([128, 512], FP32, tag="yb",
                                               name="yb", bufs=4)
                            nc.vector.tensor_copy(yb[:], pp[:])
                            nc.gpsimd.dma_start(yslc, yb[:], accum_op=ALU.add)
                        else:
                            ya = y_acc[t][:, dc * 512:(dc + 1) * 512]
                            if fc == 0:
                                nc.vector.tensor_scalar(
                                    ya, pp[:], wv[t][:], None, op0=ALU.mult
                                )
                            else:
                                nc.vector.scalar_tensor_tensor(
                                    ya, pp[:], wv[t][:], ya, op0=ALU.mult, op1=ALU.add
                                )
                            if fc == NFC - 1:
                                nc.sync.dma_start(yslc, ya)

            prev = None
            for fc in range(NFC):
                # --- gate matmuls (weights moving, N=512) ---
                wg_t = []
                for d in range(ND):
                    wgt = w_pool.tile([128, 512], MDT, tag="w", name="wtile")
                    wsrc = wg_d[d * 128:(d + 1) * 128, fc * 512:(fc + 1) * 512]
                    nc.sync.dma_start(
                        wgt[:], wsrc if use_bf16 else wsrc.bitcast(FP32R)
                    )
                    wg_t.append(wgt)
                pg = []
                for t in range(NT):
                    p = ps_pool.tile([128, 512], FP32, tag="ps", name="ps")
                    for d in range(ND):
                        nc.tensor.matmul(p[:], xt[d][:, t * 128:(t + 1) * 128],
                                         wg_t[d][:],
                                         start=(d == 0), stop=(d == ND - 1))
                    pg.append(p)
                # --- up matmuls + silu + h ---
                wu_t = []
                for d in range(ND):
                    wut = w_pool.tile([128, 512], MDT, tag="w", name="wtile")
                    wsrc = wu_d[d * 128:(d + 1) * 128, fc * 512:(fc + 1) * 512]
                    nc.sync.dma_start(
                        wut[:], wsrc if use_bf16 else wsrc.bitcast(FP32R)
                    )
                    wu_t.append(wut)
                hch = []
                for t in range(NT):
                    pu = ps_pool.tile([128, 512], FP32, tag="ps", name="ps")
                    for d in range(ND):
                        nc.tensor.matmul(pu[:], xt[d][:, t * 128:(t + 1) * 128],
                                         wu_t[d][:],
                                         start=(d == 0), stop=(d == ND - 1))
                    st = tmp_pool.tile([128, 512], FP32, tag="silu", name="silu",
                                        bufs=3 if use_bf16 else 2)
                    nc.scalar.activation(st[:], pg[t][:], ACTF.Silu)
                    hcht = hch_pool.tile([128, 512], MDT, tag="hch", name="hch")
                    if ACC_DMA:
                        nc.vector.scalar_tensor_tensor(
                            hcht[:], st[:], wv[t][:], pu[:],
                            op0=ALU.mult, op1=ALU.mult,
                        )
                    else:
                        nc.vector.tensor_mul(hcht[:], st[:], pu[:])
                    hch.append(hcht)
                # --- deferred transposes + down for the previous f-chunk ---
                if prev is not None:
                    emit_tr_down(*prev)
                prev = (fc, hch)
            emit_tr_down(*prev)

    nc.compile()
    return nc


_PROGRAM_CACHE = {}


def _get_program(C, use_bf16=USE_BF16):
    key = (C, use_bf16)
    if key not in _PROGRAM_CACHE:
        _PROGRAM_CACHE[key] = build_program(C, use_bf16)
    return _PROGRAM_CACHE[key]


def _route_host(x_TD, router_w):
    """Host dispatch: top-2 expert ids per token (selection only, no weights)."""
    logits = x_TD @ router_w  # fp32; min 2nd/3rd gap >> fp32 error
    order = np.argsort(-logits, axis=1, kind="stable")
    return order[:, :2]


def kernel_with_results(x_TD, router_w, w_gate, w_up, w_down):
    x_TD = np.ascontiguousarray(x_TD, np.float32)
    router_w = np.ascontiguousarray(router_w, np.float32)
    w_gate = np.ascontiguousarray(w_gate, np.float32)
    w_up = np.ascontiguousarray(w_up, np.float32)
    w_down = np.ascontiguousarray(w_down, np.float32)

    top2 = _route_host(x_TD, router_w)
    idx_lists = [np.where((top2 == e).any(axis=1))[0] for e in range(E)]
    max_cnt = max(len(ix) for ix in idx_lists)
    C = max(256, -(-max_cnt // 128) * 128)

    nc = _get_program(C)

    xT = np.ascontiguousarray(x_TD.T)  # [D, T]
    in_maps = []
    for e in range(E):
        ix = idx_lists[e]
        xTg = np.zeros((D, C), np.float32)
        xTg[:, :len(ix)] = xT[:, ix]
        eoh = np.zeros((1, E), np.float32)
        eoh[0, e] = 1.0
        im = {
            "xT": xTg,
            "rw": router_w,
            "eoh": eoh,
            "wg": w_gate[e] if not USE_BF16 else w_gate[e].astype(_BF16NP),
            "wu": w_up[e] if not USE_BF16 else w_up[e].astype(_BF16NP),
            "wd": w_down[e] if not USE_BF16 else w_down[e].astype(_BF16NP),
        }
        in_maps.append(im)

    try:
        res = bass_utils.run_bass_kernel_spmd(
            nc, in_maps, core_ids=list(range(NCORES))
        )
    except ModuleNotFoundError:
        # Tracing requested via env but the axon NTFF hook module is absent
        # in this image — rerun without tracing.
        os.environ["BASS_NEVER_TRACE"] = "1"
        res = bass_utils.run_bass_kernel_spmd(
            nc, in_maps, core_ids=list(range(NCORES))
        )

    out = np.zeros((T, D), np.float32)
    for e in range(E):
        ix = idx_lists[e]
        y = res.results[e]["y"]  # [C, D]
        out[ix] += y[:len(ix)]
    return out, res


def kernel(**inputs):
    out, _ = kernel_with_results(**inputs)
    return out



# revision 2
# speedup vs baseline: 1.1964x; 1.1964x over previous
# MoE top-2 routing kernel for 8 Trainium2 NeuronCores (expert-parallel).
#
# Problem (hardcoded shapes): T=2048 tokens, D=2048 model dim, F=4096 ffn dim,
# E=8 experts, top-2 routing with renormalized softmax weights.
#
# Sharding: one expert per core. The host does the router (fp32 logits ->
# top-2 selection + renormalized softmax weights; selection is numerically
# unambiguous: min 2nd-vs-3rd prob gap ~9e-5, orders of magnitude above fp32
# matmul noise), gathers each expert's tokens into a transposed buffer
# xb [D, C] (C = max expert load rounded up to 32), and zero-pads the tail.
# Zero columns are harmless: MLP(0) = 0 and the host ignores pad columns.
#
# Device structure (all matmuls weights-STATIONARY, tokens moving): PE cost
# scales with the actual token count C (544 here) instead of 128-padded
# token tiles (5*128=640 in the v1 layout), and gate/up naturally produce
# [f, t] layout so the down matmul needs no PE transposes at all.
#   gate/up: pg[f128, C] = sum_d wg[d,f128].T @ x[d, C]   (per 128-f tile)
#   h[f, t] = silu(g) * u  (scalar+vector, bf16)
#   down:    y[d128, C] += sum_f wd[f,d128].T @ h[f, C]   (PSUM per 8-f group,
#            accumulated into SBUF ya, router-weight scaled, DMA'd out [D, C])
# Moving-dim chunks of C/2=272 <= 512 fp32 PSUM bank columns. bf16 matmuls
# stream ~1 col/cycle; FWL hides the 128-col LDWEIGHTS under 272-col matmuls.

import os
import numpy as np
import ml_dtypes

_BF16NP = ml_dtypes.bfloat16

import concourse.bass as bass
import concourse.bacc as bacc
import concourse.mybir as mybir
import concourse.tile as tile
from concourse import bass_utils

FP32 = mybir.dt.float32
BF16 = mybir.dt.bfloat16
AX = mybir.AxisListType
ALU = mybir.AluOpType
ACTF = mybir.ActivationFunctionType

T, D, F, E = 2048, 2048, 4096, 8
NCORES = 8
ND = D // 128    # 16 d-tiles (contraction for gate/up; output tiles for down)
NF = F // 128    # 32 f-tiles
G = 8            # f-tiles per down-accumulation group
NG = NF // G     # 4 groups


def _chunks16(C):
    """Split C token columns into PSUM-bank chunks (<=512 fp32, mult of 16)."""
    nch = (C + 511) // 512
    out, rem, c0 = [], C, 0
    for i in range(nch):
        cn = -(-(rem // (nch - i)) // 16) * 16
        cn = min(cn, rem)
        out.append((c0, cn))
        c0 += cn
        rem -= cn
    return out


def build_program(C):
    chunks = _chunks16(C)
    nc = bacc.Bacc(
        "TRN2",
        target_bir_lowering=False,
        debug=False,
        enable_asserts=False,
        num_devices=NCORES,
    )
    xb_d = nc.dram_tensor("xb", [D, C], BF16, kind="ExternalInput").ap()
    wvb_d = nc.dram_tensor("wvb", [128, C], FP32, kind="ExternalInput").ap()
    wg_d = nc.dram_tensor("wg", [D, F], BF16, kind="ExternalInput").ap()
    wu_d = nc.dram_tensor("wu", [D, F], BF16, kind="ExternalInput").ap()
    wd_d = nc.dram_tensor("wd", [F, D], BF16, kind="ExternalInput").ap()
    y_d = nc.dram_tensor("y", [D, C], FP32, kind="ExternalOutput").ap()

    with tile.TileContext(nc) as tc:
        with (
            tc.tile_pool(name="const", bufs=1) as const_pool,
            tc.tile_pool(name="x", bufs=1) as x_pool,
            tc.tile_pool(name="ya", bufs=1) as ya_pool,
            tc.tile_pool(name="h", bufs=2) as h_pool,
            tc.tile_pool(name="wgu", bufs=48) as wgu_pool,
            tc.tile_pool(name="wdp", bufs=2) as wd_pool,
            tc.tile_pool(name="tmp", bufs=2) as tmp_pool,
            tc.tile_pool(name="ps", bufs=6, space="PSUM") as ps_pool,
            tc.tile_pool(name="psy", bufs=2, space="PSUM") as psy_pool,
        ):
            # ---- resident inputs ----
            wvb = const_pool.tile([128, C], FP32, tag="wvb", name="wvb")
            nc.sync.dma_start(wvb[:], wvb_d[:])
            xt = [x_pool.tile([128, C], BF16, tag=f"xt{d}", name=f"xt{d}")
                  for d in range(ND)]
            for d in range(ND):
                nc.sync.dma_start(xt[d][:], xb_d[d * 128:(d + 1) * 128, :])
            ya = [ya_pool.tile([128, C], FP32, tag=f"ya{dt}", name=f"ya{dt}")
                  for dt in range(ND)]

            wg_sb = {}   # fc -> list of 16 [128, 512] tiles
            wu_sb = {}
            wd_sb = {}   # ft -> [128, 2048] tile

            def emit_down(gprev, hprev, j):
                """Down-matmul (d-tiles 2j, 2j+1) for f-group gprev."""
                f0 = gprev * G
                for dt in (2 * j, 2 * j + 1):
                    for (c0, cn) in chunks:
                        py = psy_pool.tile([128, max(cn for _, cn in chunks)],
                                           FP32, tag="py", name="py")
                        for k in range(G):
                            nc.tensor.matmul(
                                py[:, :cn],
                                wd_sb[f0 + k][:, dt * 128:(dt + 1) * 128],
                                hprev[k][:, c0:c0 + cn],
                                start=(k == 0), stop=(k == G - 1),
                            )
                        yslc = ya[dt][:, c0:c0 + cn]
                        if gprev == 0:
                            nc.scalar.copy(yslc, py[:, :cn])
                        else:
                            nc.vector.tensor_tensor(yslc, yslc, py[:, :cn],
                                                    op=ALU.add)
                    if gprev == NG - 1:
                        nc.vector.tensor_mul(ya[dt][:], ya[dt][:], wvb[:])
                        nc.sync.dma_start(
                            y_d[dt * 128:(dt + 1) * 128, :], ya[dt][:])

            hprev = None
            for g in range(NG):
                hcur = []
                for j in range(G):
                    ft = g * G + j
                    fc, fo = divmod(ft, 4)
                    if fo == 0:
                        wg_sb[fc] = []
                        wu_sb[fc] = []
                        for d in range(ND):
                            wgt = wgu_pool.tile([128, 512], BF16, tag="w",
                                                name="wgt")
                            nc.sync.dma_start(
                                wgt[:],
                                wg_d[d * 128:(d + 1) * 128,
                                     fc * 512:(fc + 1) * 512])
                            wg_sb[fc].append(wgt)
                        for d in range(ND):
                            wut = wgu_pool.tile([128, 512], BF16, tag="w",
                                                name="wut")
                            nc.sync.dma_start(
                                wut[:],
                                wu_d[d * 128:(d + 1) * 128,
                                     fc * 512:(fc + 1) * 512])
                            wu_sb[fc].append(wut)
                    # wd tile for this f-tile (used by down in next group)
                    wdt = wd_pool.tile([128, D], BF16, tag=f"wd{j}", name="wdt")
                    nc.sync.dma_start(wdt[:], wd_d[ft * 128:(ft + 1) * 128, :])
                    wd_sb[ft] = wdt

                    fsl = slice(fo * 128, (fo + 1) * 128)
                    # gate -> pg
                    pg = [ps_pool.tile([128, cn], FP32, tag="ps", name="ps")
                          for (c0, cn) in chunks]
                    for d in range(ND):
                        for ci, (c0, cn) in enumerate(chunks):
                            nc.tensor.matmul(
                                pg[ci][:], wg_sb[fc][d][:, fsl],
                                xt[d][:, c0:c0 + cn],
                                start=(d == 0), stop=(d == ND - 1),
                            )
                    st = tmp_pool.tile([128, C], FP32, tag="st", name="st")
                    for ci, (c0, cn) in enumerate(chunks):
                        nc.scalar.activation(st[:, c0:c0 + cn], pg[ci][:],
                                             ACTF.Silu)
                    # up -> pu
                    pu = [ps_pool.tile([128, cn], FP32, tag="ps", name="ps")
                          for (c0, cn) in chunks]
                    for d in range(ND):
                        for ci, (c0, cn) in enumerate(chunks):
                            nc.tensor.matmul(
                                pu[ci][:], wu_sb[fc][d][:, fsl],
                                xt[d][:, c0:c0 + cn],
                                start=(d == 0), stop=(d == ND - 1),
                            )
                    ht = h_pool.tile([128, C], BF16, tag=f"h{j}", name=f"h{j}")
                    for ci, (c0, cn) in enumerate(chunks):
                        nc.vector.tensor_mul(ht[:, c0:c0 + cn],
                                             st[:, c0:c0 + cn], pu[ci][:])
                    hcur.append(ht)
                    if hprev is not None:
                        emit_down(g - 1, hprev, j)
                hprev = hcur
            for j in range(G):
                emit_down(NG - 1, hprev, j)

    nc.compile()
    return nc


_PROGRAM_CACHE = {}


def _get_program(C):
    if C not in _PROGRAM_CACHE:
        _PROGRAM_CACHE[C] = build_program(C)
    return _PROGRAM_CACHE[C]


def _route_host(x_TD, router_w):
    """Host router: top-2 ids + renormalized softmax weights per token."""
    logits = (x_TD @ router_w).astype(np.float64)  # [T, E]
    logits -= logits.max(axis=1, keepdims=True)
    probs = np.exp(logits)
    probs /= probs.sum(axis=1, keepdims=True)
    order = np.argsort(-probs, axis=1, kind="stable")
    top2 = order[:, :2]
    w12 = np.take_along_axis(probs, top2, axis=1)
    w12 /= w12.sum(axis=1, keepdims=True)
    return top2, w12.astype(np.float32)


def kernel_with_results(x_TD, router_w, w_gate, w_up, w_down):
    x_TD = np.ascontiguousarray(x_TD, np.float32)
    router_w = np.ascontiguousarray(router_w, np.float32)

    top2, w12 = _route_host(x_TD, router_w)
    idx_lists, wt_lists = [], []
    for e in range(E):
        sel = top2 == e  # [T, 2]
        ix = np.where(sel.any(axis=1))[0]
        idx_lists.append(ix)
        wt_lists.append(w12[sel.any(axis=1), :][sel[ix]])
    max_cnt = max(len(ix) for ix in idx_lists)
    C = max(64, -(-max_cnt // 32) * 32)

    nc = _get_program(C)

    xT = np.ascontiguousarray(x_TD.T).astype(_BF16NP)  # [D, T] bf16
    in_maps = []
    for e in range(E):
        ix = idx_lists[e]
        xb = np.zeros((D, C), _BF16NP)
        xb[:, :len(ix)] = xT[:, ix]
        wvb = np.zeros((1, C), np.float32)
        wvb[0, :len(ix)] = wt_lists[e]
        in_maps.append({
            "xb": xb,
            "wvb": np.ascontiguousarray(np.broadcast_to(wvb, (128, C))),
            "wg": w_gate[e].astype(_BF16NP),
            "wu": w_up[e].astype(_BF16NP),
            "wd": w_down[e].astype(_BF16NP),
        })

    try:
        res = bass_utils.run_bass_kernel_spmd(
            nc, in_maps, core_ids=list(range(NCORES))
        )
    except ModuleNotFoundError:
        # Tracing requested via env but the axon NTFF hook module is absent
        # in this image — rerun without tracing.
        os.environ["BASS_NEVER_TRACE"] = "1"
        res = bass_utils.run_bass_kernel_spmd(
            nc, in_maps, core_ids=list(range(NCORES))
        )

    out = np.zeros((T, D), np.float32)
    for e in range(E):
        ix = idx_lists[e]
        y = res.results[e]["y"]  # [D, C]
        out[ix] += y[:, :len(ix)].T
    return out, res


def kernel(**inputs):
    out, _ = kernel_with_results(**inputs)
    return out


# revision 4
# speedup vs baseline: 1.2340x; 1.0314x over previous
# MoE top-2 routing kernel for 8 Trainium2 NeuronCores (expert-parallel).
#
# Problem (hardcoded shapes): T=2048 tokens, D=2048 model dim, F=4096 ffn dim,
# E=8 experts, top-2 routing with renormalized softmax weights.
#
# Sharding: one expert per core. The host does the router (fp32 logits ->
# top-2 selection + renormalized softmax weights; selection is numerically
# unambiguous: min 2nd-vs-3rd prob gap ~9e-5, orders of magnitude above fp32
# matmul noise), gathers each expert's tokens into a transposed buffer
# xb [D, C] (C = max expert load rounded up to 8), and zero-pads the tail.
# Zero columns are harmless: MLP(0) = 0 and the host ignores pad columns.
#
# Device structure (all matmuls weights-STATIONARY, tokens moving): PE cost
# scales with the actual token count C (536 here) instead of 128-padded
# token tiles (5*128=640 in the v1 layout), and gate/up naturally produce
# [f, t] layout so the down matmul needs no PE transposes at all.
#   gate/up: pg[f128, C] = sum_d wg[d,f128].T @ x[d, C]   (per 128-f tile)
#   h[f, t] = silu(g) * u  (scalar+vector, bf16)
#   down:    y[d128, C] += sum_f wd[f,d128].T @ h[f, C]   (PSUM per 8-f group,
#            accumulated into SBUF ya, router-weight scaled, DMA'd out [D, C])
# Moving-dim chunks of ~C/2 <= 512 fp32 PSUM bank columns. bf16 matmuls
# stream ~1 col/cycle at the 2.0 GHz PE clock; LDWEIGHTS pipelines under the
# matmuls via the PE reorder window (trace: 139 ns per 272-col matmul, zero
# scheduling stalls).
#
# DMA: each DMA instruction costs ~700ns of issue on its queue engine and
# carries ~128 descriptors in parallel across 16 HW engines, so small tiles
# cap aggregate bandwidth (~175 GB/s at 128KB/instr). Weights load as ONE
# instruction per 512-f-chunk ([128, 16*512] rearranged), x as 4, and the
# wu/wd/wvb loads are emitted after the first gate matmuls so the startup
# critical path is just x + the first gate weight chunk.

import os
import numpy as np
import ml_dtypes

_BF16NP = ml_dtypes.bfloat16

import concourse.bass as bass
import concourse.bacc as bacc
import concourse.mybir as mybir
import concourse.tile as tile
from concourse import bass_utils

FP32 = mybir.dt.float32
BF16 = mybir.dt.bfloat16
AX = mybir.AxisListType
ALU = mybir.AluOpType
ACTF = mybir.ActivationFunctionType

T, D, F, E = 2048, 2048, 4096, 8
NCORES = 8
ND = D // 128    # 16 d-tiles (contraction for gate/up; output tiles for down)
NF = F // 128    # 32 f-tiles
G = 8            # f-tiles per down-accumulation group
NG = NF // G     # 4 groups


def _chunks8(C):
    """Split C token columns into PSUM-bank chunks (<=512 fp32, mult of 8)."""
    nch = (C + 511) // 512
    out, rem, c0 = [], C, 0
    for i in range(nch):
        cn = -(-(rem // (nch - i)) // 8) * 8
        cn = min(cn, rem)
        out.append((c0, cn))
        c0 += cn
        rem -= cn
    return out


def build_program(C):
    chunks = _chunks8(C)
    nc = bacc.Bacc(
        "TRN2",
        target_bir_lowering=False,
        debug=False,
        enable_asserts=False,
        num_devices=NCORES,
    )
    xb_d = nc.dram_tensor("xb", [D, C], BF16, kind="ExternalInput").ap()
    wvb_d = nc.dram_tensor("wvb", [128, C], FP32, kind="ExternalInput").ap()
    wg_d = nc.dram_tensor("wg", [D, F], BF16, kind="ExternalInput").ap()
    wu_d = nc.dram_tensor("wu", [D, F], BF16, kind="ExternalInput").ap()
    wd_d = nc.dram_tensor("wd", [F, D], BF16, kind="ExternalInput").ap()
    y_d = nc.dram_tensor("y", [D, C], FP32, kind="ExternalOutput").ap()

    with tile.TileContext(nc) as tc:
        with (
            tc.tile_pool(name="const", bufs=1) as const_pool,
            tc.tile_pool(name="x", bufs=1) as x_pool,
            tc.tile_pool(name="ya", bufs=1) as ya_pool,
            tc.tile_pool(name="h", bufs=2) as h_pool,
            tc.tile_pool(name="wgu", bufs=4) as wgu_pool,
            tc.tile_pool(name="wdp", bufs=2) as wd_pool,
            tc.tile_pool(name="tmp", bufs=2) as tmp_pool,
            tc.tile_pool(name="ps", bufs=6, space="PSUM") as ps_pool,
            tc.tile_pool(name="psy", bufs=2, space="PSUM") as psy_pool,
        ):
            # ---- x residency: 4 merged DMAs of 4 d-tiles each ----
            xt4 = [x_pool.tile([128, 4 * C], BF16, tag=f"xt{q}", name=f"xt{q}")
                   for q in range(4)]
            for q in range(4):
                nc.sync.dma_start(
                    xt4[q][:].rearrange("p (n c) -> p n c", n=4),
                    xb_d[q * 512:(q + 1) * 512, :]
                    .rearrange("(n p) c -> p n c", p=128),
                )

            def xsl(d, c0, cn):
                q, r = divmod(d, 4)
                return xt4[q][:, r * C + c0:r * C + c0 + cn]

            wvb = const_pool.tile([128, C], FP32, tag="wvb", name="wvb")
            ya = [ya_pool.tile([128, C], FP32, tag=f"ya{dt}", name=f"ya{dt}")
                  for dt in range(ND)]

            wg_sb = {}   # fc -> [128, 16*512] tile (all 16 d-tiles merged)
            wu_sb = {}
            wd_sb = {}   # ft -> [128, 2048] tile

            def load_w(dram, fc, name):
                wt = wgu_pool.tile([128, ND * 512], BF16, tag="w", name=name)
                nc.sync.dma_start(
                    wt[:].rearrange("p (n f) -> p n f", n=ND),
                    dram[:, fc * 512:(fc + 1) * 512]
                    .rearrange("(n p) f -> p n f", p=128),
                )
                return wt

            def emit_down(gprev, hprev, j):
                """Down-matmul (d-tiles 2j, 2j+1) for f-group gprev."""
                f0 = gprev * G
                for dt in (2 * j, 2 * j + 1):
                    for (c0, cn) in chunks:
                        py = psy_pool.tile([128, max(cn for _, cn in chunks)],
                                           FP32, tag="py", name="py")
                        for k in range(G):
                            nc.tensor.matmul(
                                py[:, :cn],
                                wd_sb[f0 + k][:, dt * 128:(dt + 1) * 128],
                                hprev[k][:, c0:c0 + cn],
                                start=(k == 0), stop=(k == G - 1),
                            )
                        yslc = ya[dt][:, c0:c0 + cn]
                        if gprev == 0:
                            nc.scalar.copy(yslc, py[:, :cn])
                        else:
                            nc.vector.tensor_tensor(yslc, yslc, py[:, :cn],
                                                    op=ALU.add)
                    if gprev == NG - 1:
                        nc.vector.tensor_mul(ya[dt][:], ya[dt][:], wvb[:])
                        nc.sync.dma_start(
                            y_d[dt * 128:(dt + 1) * 128, :], ya[dt][:])

            hprev = None
            for g in range(NG):
                hcur = []
                for j in range(G):
                    ft = g * G + j
                    fc, fo = divmod(ft, 4)
                    if fo == 0:
                        wg_sb[fc] = load_w(wg_d, fc, "wgt")
                    # gate -> pg
                    pg = [ps_pool.tile([128, cn], FP32, tag="ps", name="ps")
                          for (c0, cn) in chunks]
                    for d in range(ND):
                        wsl = wg_sb[fc][:, d * 512 + fo * 128:
                                        d * 512 + (fo + 1) * 128]
                        for ci, (c0, cn) in enumerate(chunks):
                            nc.tensor.matmul(
                                pg[ci][:], wsl, xsl(d, c0, cn),
                                start=(d == 0), stop=(d == ND - 1),
                            )
                    # up-weights / wvb DMAs go behind the first gate matmuls
                    if fo == 0:
                        wu_sb[fc] = load_w(wu_d, fc, "wut")
                    if g == NG - 1 and j == 0:
                        nc.sync.dma_start(wvb[:], wvb_d[:])
                    st = tmp_pool.tile([128, C], FP32, tag="st", name="st")
                    for ci, (c0, cn) in enumerate(chunks):
                        nc.scalar.activation(st[:, c0:c0 + cn], pg[ci][:],
                                             ACTF.Silu)
                    # up -> pu
                    pu = [ps_pool.tile([128, cn], FP32, tag="ps", name="ps")
                          for (c0, cn) in chunks]
                    for d in range(ND):
                        wsl = wu_sb[fc][:, d * 512 + fo * 128:
                                        d * 512 + (fo + 1) * 128]
                        for ci, (c0, cn) in enumerate(chunks):
                            nc.tensor.matmul(
                                pu[ci][:], wsl, xsl(d, c0, cn),
                                start=(d == 0), stop=(d == ND - 1),
                            )
                    ht = h_pool.tile([128, C], BF16, tag=f"h{j}", name=f"h{j}")
                    for ci, (c0, cn) in enumerate(chunks):
                        nc.vector.tensor_mul(ht[:, c0:c0 + cn],
                                             st[:, c0:c0 + cn], pu[ci][:])
                    hcur.append(ht)
                    # down-weights for this f-tile (used next group)
                    wdt = wd_pool.tile([128, D], BF16, tag=f"wd{j}", name="wdt")
                    nc.sync.dma_start(wdt[:], wd_d[ft * 128:(ft + 1) * 128, :])
                    wd_sb[ft] = wdt
                    if hprev is not None:
                        emit_down(g - 1, hprev, j)
                hprev = hcur
            for j in range(G):
                emit_down(NG - 1, hprev, j)

    nc.compile()
    return nc


_PROGRAM_CACHE = {}


def _get_program(C):
    if C not in _PROGRAM_CACHE:
        _PROGRAM_CACHE[C] = build_program(C)
    return _PROGRAM_CACHE[C]


def _route_host(x_TD, router_w):
    """Host router: top-2 ids + renormalized softmax weights per token."""
    logits = (x_TD @ router_w).astype(np.float64)  # [T, E]
    logits -= logits.max(axis=1, keepdims=True)
    probs = np.exp(logits)
    probs /= probs.sum(axis=1, keepdims=True)
    order = np.argsort(-probs, axis=1, kind="stable")
    top2 = order[:, :2]
    w12 = np.take_along_axis(probs, top2, axis=1)
    w12 /= w12.sum(axis=1, keepdims=True)
    return top2, w12.astype(np.float32)


def kernel_with_results(x_TD, router_w, w_gate, w_up, w_down):
    x_TD = np.ascontiguousarray(x_TD, np.float32)
    router_w = np.ascontiguousarray(router_w, np.float32)

    top2, w12 = _route_host(x_TD, router_w)
    idx_lists, wt_lists = [], []
    for e in range(E):
        sel = top2 == e  # [T, 2]
        any_sel = sel.any(axis=1)
        ix = np.where(any_sel)[0]
        idx_lists.append(ix)
        wt_lists.append(w12[any_sel][sel[ix]])
    max_cnt = max(len(ix) for ix in idx_lists)
    C = max(64, -(-max_cnt // 8) * 8)

    nc = _get_program(C)

    xT = np.ascontiguousarray(x_TD.T).astype(_BF16NP)  # [D, T] bf16
    in_maps = []
    for e in range(E):
        ix = idx_lists[e]
        xb = np.zeros((D, C), _BF16NP)
        xb[:, :len(ix)] = xT[:, ix]
        wvb = np.zeros((1, C), np.float32)
        wvb[0, :len(ix)] = wt_lists[e]
        in_maps.append({
            "xb": xb,
            "wvb": np.ascontiguousarray(np.broadcast_to(wvb, (128, C))),
            "wg": w_gate[e].astype(_BF16NP),
            "wu": w_up[e].astype(_BF16NP),
            "wd": w_down[e].astype(_BF16NP),
        })

    try:
        res = bass_utils.run_bass_kernel_spmd(
            nc, in_maps, core_ids=list(range(NCORES))
        )
    except ModuleNotFoundError:
        # Tracing requested via env but the axon NTFF hook module is absent
        # in this image — rerun without tracing.
        os.environ["BASS_NEVER_TRACE"] = "1"
        res = bass_utils.run_bass_kernel_spmd(
            nc, in_maps, core_ids=list(range(NCORES))
        )

    out = np.zeros((T, D), np.float32)
    for e in range(E):
        ix = idx_lists[e]
        y = res.results[e]["y"]  # [D, C]
        out[ix] += y[:, :len(ix)].T
    return out, res


def kernel(**inputs):
    out, _ = kernel_with_results(**inputs)
    return out


# revision 7
# speedup vs baseline: 1.4614x; 1.1842x over previous
# MoE top-2 routing kernel for 8 Trainium2 NeuronCores (expert-parallel).
#
# Problem (hardcoded shapes): T=2048 tokens, D=2048 model dim, F=4096 ffn dim,
# E=8 experts, top-2 routing with renormalized softmax weights.
#
# Sharding: one expert per core. The host does the router (fp32 logits ->
# top-2 selection + renormalized softmax weights; selection is numerically
# unambiguous: min 2nd-vs-3rd prob gap ~9e-5, orders of magnitude above fp32
# matmul noise), gathers each expert's tokens into a transposed buffer
# xb [D, C] (C = max expert load rounded up to 8), and zero-pads the tail.
# Zero columns are harmless: MLP(0) = 0 and the host ignores pad columns.
#
# Device structure (all matmuls weights-STATIONARY, tokens moving): PE cost
# scales with the actual token count C (536 here) instead of 128-padded
# token tiles (5*128=640 in the v1 layout), and gate/up naturally produce
# [f, t] layout so the down matmul needs no PE transposes at all.
#   gate/up: pg[f128, C] = sum_d wg[d,f128].T @ x[d, C]   (per 128-f tile)
#   h[f, t] = silu(g) * u  (scalar+vector, bf16)
#   down:    y[d128, C] += sum_f wd[f,d128].T @ h[f, C]   (PSUM per 8-f group,
#            accumulated into SBUF ya, router-weight scaled, DMA'd out [D, C])
# Moving-dim chunks of ~C/2 <= 512 fp32 PSUM bank columns. bf16 matmuls
# stream ~1 col/cycle at the 2.0 GHz PE clock; LDWEIGHTS pipelines under the
# matmuls via the PE reorder window (trace: 139 ns per 272-col matmul, zero
# scheduling stalls).
#
# DMA: each DMA instruction costs ~700ns of issue on its queue engine and
# carries ~128 descriptors in parallel across 16 HW engines, so small tiles
# cap aggregate bandwidth (~175 GB/s at 128KB/instr). Weights load as ONE
# instruction per 512-f-chunk ([128, 16*512] rearranged), x as 4, and the
# wu/wd/wvb loads are emitted after the first gate matmuls so the startup
# critical path is just x + the first gate weight chunk.

import os
import numpy as np
import ml_dtypes

_BF16NP = ml_dtypes.bfloat16

import concourse.bass as bass
import concourse.bacc as bacc
import concourse.mybir as mybir
import concourse.tile as tile
from concourse import bass_utils

FP32 = mybir.dt.float32
BF16 = mybir.dt.bfloat16
AX = mybir.AxisListType
ALU = mybir.AluOpType
ACTF = mybir.ActivationFunctionType

T, D, F, E = 2048, 2048, 4096, 8
NCORES = 8
ND = D // 128    # 16 d-tiles (contraction for gate/up; output tiles for down)
NF = F // 128    # 32 f-tiles
G = 8            # f-tiles per down-accumulation group
NG = NF // G     # 4 groups


def _chunks8(C):
    """Split C token columns into PSUM-bank chunks (<=512 fp32, mult of 8)."""
    nch = (C + 511) // 512
    out, rem, c0 = [], C, 0
    for i in range(nch):
        cn = -(-(rem // (nch - i)) // 8) * 8
        cn = min(cn, rem)
        out.append((c0, cn))
        c0 += cn
        rem -= cn
    return out


def build_program(C):
    chunks = _chunks8(C)
    nc = bacc.Bacc(
        "TRN2",
        target_bir_lowering=False,
        debug=False,
        enable_asserts=False,
        num_devices=NCORES,
    )
    xb_d = nc.dram_tensor("xb", [D, C], BF16, kind="ExternalInput").ap()
    wvb_d = nc.dram_tensor("wvb", [128, C], FP32, kind="ExternalInput").ap()
    wg_d = nc.dram_tensor("wg", [D, F], BF16, kind="ExternalInput").ap()
    wu_d = nc.dram_tensor("wu", [D, F], BF16, kind="ExternalInput").ap()
    wd_d = nc.dram_tensor("wd", [F, D], BF16, kind="ExternalInput").ap()
    y_d = nc.dram_tensor("y", [D, C], FP32, kind="ExternalOutput").ap()

    with tile.TileContext(nc) as tc:
        with (
            tc.tile_pool(name="const", bufs=1) as const_pool,
            tc.tile_pool(name="x", bufs=1) as x_pool,
            tc.tile_pool(name="ya", bufs=1) as ya_pool,
            tc.tile_pool(name="h", bufs=2) as h_pool,
            tc.tile_pool(name="wgu", bufs=4) as wgu_pool,
            tc.tile_pool(name="wdp", bufs=2) as wd_pool,
            tc.tile_pool(name="tmp", bufs=2) as tmp_pool,
            tc.tile_pool(name="ps", bufs=6, space="PSUM") as ps_pool,
            tc.tile_pool(name="psy", bufs=2, space="PSUM") as psy_pool,
        ):
            # ---- PE warm-up: dummy matmuls on memset data while the first
            # DMAs stream in, so the HAM clock-gate reaches 8/8 (full rate)
            # before the real matmuls start. PE is otherwise idle here. ----
            warm = const_pool.tile([128, 640], BF16, tag="warm", name="warm")
            nc.vector.memset(warm[:], 0.0)
            for _ in range(12):
                pw = ps_pool.tile([128, 512], FP32, tag="ps", name="ps")
                nc.tensor.matmul(pw[:], warm[:, :128], warm[:, 128:640],
                                 start=True, stop=True)

            # ---- first gate weights lead the DMA queue (split in halves so
            # the first matmuls can start after ~1MB), then x residency ----
            wg_sb = {}   # fc -> [128, 16*512] tile (all 16 d-tiles merged)
            wu_sb = {}
            wd_sb = {}   # ft -> [128, 2048] tile

            wg_sb[0] = wgu_pool.tile([128, ND * 512], BF16, tag="w",
                                     name="wgt")
            for half in range(2):
                nc.sync.dma_start(
                    wg_sb[0][:, half * 8 * 512:(half + 1) * 8 * 512]
                    .rearrange("p (n f) -> p n f", n=8),
                    wg_d[half * 1024:(half + 1) * 1024, :512]
                    .rearrange("(n p) f -> p n f", p=128),
                )

            xt4 = [x_pool.tile([128, 4 * C], BF16, tag=f"xt{q}", name=f"xt{q}")
                   for q in range(4)]
            for q in range(4):
                nc.sync.dma_start(
                    xt4[q][:].rearrange("p (n c) -> p n c", n=4),
                    xb_d[q * 512:(q + 1) * 512, :]
                    .rearrange("(n p) c -> p n c", p=128),
                )

            def xsl(d, c0, cn):
                q, r = divmod(d, 4)
                return xt4[q][:, r * C + c0:r * C + c0 + cn]

            wvb = const_pool.tile([128, C], FP32, tag="wvb", name="wvb")
            ya = [ya_pool.tile([128, C], FP32, tag=f"ya{dt}", name=f"ya{dt}")
                  for dt in range(ND)]

            def load_w(dram, fc, name):
                wt = wgu_pool.tile([128, ND * 512], BF16, tag="w", name=name)
                nc.sync.dma_start(
                    wt[:].rearrange("p (n f) -> p n f", n=ND),
                    dram[:, fc * 512:(fc + 1) * 512]
                    .rearrange("(n p) f -> p n f", p=128),
                )
                return wt

            def emit_down(gprev, hprev, j):
                """Down-matmul (d-tiles 2j, 2j+1) for f-group gprev."""
                f0 = gprev * G
                for dt in (2 * j, 2 * j + 1):
                    for (c0, cn) in chunks:
                        py = psy_pool.tile([128, max(cn for _, cn in chunks)],
                                           FP32, tag="py", name="py")
                        for k in range(G):
                            nc.tensor.matmul(
                                py[:, :cn],
                                wd_sb[f0 + k][:, dt * 128:(dt + 1) * 128],
                                hprev[k][:, c0:c0 + cn],
                                start=(k == 0), stop=(k == G - 1),
                            )
                        yslc = ya[dt][:, c0:c0 + cn]
                        if gprev == 0:
                            nc.scalar.copy(yslc, py[:, :cn])
                        else:
                            nc.vector.tensor_tensor(yslc, yslc, py[:, :cn],
                                                    op=ALU.add)
                    if gprev == NG - 1:
                        nc.vector.tensor_mul(ya[dt][:], ya[dt][:], wvb[:])
                        nc.sync.dma_start(
                            y_d[dt * 128:(dt + 1) * 128, :], ya[dt][:])

            hprev = None
            for g in range(NG):
                hcur = []
                for j in range(G):
                    ft = g * G + j
                    fc, fo = divmod(ft, 4)
                    if fo == 0 and fc > 0:
                        wg_sb[fc] = load_w(wg_d, fc, "wgt")
                    # gate -> pg
                    pg = [ps_pool.tile([128, cn], FP32, tag="ps", name="ps")
                          for (c0, cn) in chunks]
                    for d in range(ND):
                        wsl = wg_sb[fc][:, d * 512 + fo * 128:
                                        d * 512 + (fo + 1) * 128]
                        for ci, (c0, cn) in enumerate(chunks):
                            nc.tensor.matmul(
                                pg[ci][:], wsl, xsl(d, c0, cn),
                                start=(d == 0), stop=(d == ND - 1),
                            )
                    # up-weights / wvb DMAs go behind the first gate matmuls
                    if fo == 0:
                        wu_sb[fc] = load_w(wu_d, fc, "wut")
                    if g == NG - 1 and j == 0:
                        nc.sync.dma_start(wvb[:], wvb_d[:])
                    st = tmp_pool.tile([128, C], FP32, tag="st", name="st")
                    for ci, (c0, cn) in enumerate(chunks):
                        nc.scalar.activation(st[:, c0:c0 + cn], pg[ci][:],
                                             ACTF.Silu)
                    # up -> pu
                    pu = [ps_pool.tile([128, cn], FP32, tag="ps", name="ps")
                          for (c0, cn) in chunks]
                    for d in range(ND):
                        wsl = wu_sb[fc][:, d * 512 + fo * 128:
                                        d * 512 + (fo + 1) * 128]
                        for ci, (c0, cn) in enumerate(chunks):
                            nc.tensor.matmul(
                                pu[ci][:], wsl, xsl(d, c0, cn),
                                start=(d == 0), stop=(d == ND - 1),
                            )
                    ht = h_pool.tile([128, C], BF16, tag=f"h{j}", name=f"h{j}")
                    for ci, (c0, cn) in enumerate(chunks):
                        nc.vector.tensor_mul(ht[:, c0:c0 + cn],
                                             st[:, c0:c0 + cn], pu[ci][:])
                    hcur.append(ht)
                    # down-weights for this f-tile (used next group)
                    wdt = wd_pool.tile([128, D], BF16, tag=f"wd{j}", name="wdt")
                    nc.sync.dma_start(wdt[:], wd_d[ft * 128:(ft + 1) * 128, :])
                    wd_sb[ft] = wdt
                    if hprev is not None:
                        emit_down(g - 1, hprev, j)
                hprev = hcur
            for j in range(G):
                emit_down(NG - 1, hprev, j)

    nc.compile()
    return nc


_PROGRAM_CACHE = {}


def _get_program(C):
    if C not in _PROGRAM_CACHE:
        _PROGRAM_CACHE[C] = build_program(C)
    return _PROGRAM_CACHE[C]


def _route_host(x_TD, router_w):
    """Host router: top-2 ids + renormalized softmax weights per token."""
    logits = (x_TD @ router_w).astype(np.float64)  # [T, E]
    logits -= logits.max(axis=1, keepdims=True)
    probs = np.exp(logits)
    probs /= probs.sum(axis=1, keepdims=True)
    order = np.argsort(-probs, axis=1, kind="stable")
    top2 = order[:, :2]
    w12 = np.take_along_axis(probs, top2, axis=1)
    w12 /= w12.sum(axis=1, keepdims=True)
    return top2, w12.astype(np.float32)


def kernel_with_results(x_TD, router_w, w_gate, w_up, w_down):
    x_TD = np.ascontiguousarray(x_TD, np.float32)
    router_w = np.ascontiguousarray(router_w, np.float32)

    top2, w12 = _route_host(x_TD, router_w)
    idx_lists, wt_lists = [], []
    for e in range(E):
        sel = top2 == e  # [T, 2]
        any_sel = sel.any(axis=1)
        ix = np.where(any_sel)[0]
        idx_lists.append(ix)
        wt_lists.append(w12[any_sel][sel[ix]])
    max_cnt = max(len(ix) for ix in idx_lists)
    C = max(64, -(-max_cnt // 8) * 8)

    nc = _get_program(C)

    xT = np.ascontiguousarray(x_TD.T).astype(_BF16NP)  # [D, T] bf16
    in_maps = []
    for e in range(E):
        ix = idx_lists[e]
        xb = np.zeros((D, C), _BF16NP)
        xb[:, :len(ix)] = xT[:, ix]
        wvb = np.zeros((1, C), np.float32)
        wvb[0, :len(ix)] = wt_lists[e]
        in_maps.append({
            "xb": xb,
            "wvb": np.ascontiguousarray(np.broadcast_to(wvb, (128, C))),
            "wg": w_gate[e].astype(_BF16NP),
            "wu": w_up[e].astype(_BF16NP),
            "wd": w_down[e].astype(_BF16NP),
        })

    try:
        res = bass_utils.run_bass_kernel_spmd(
            nc, in_maps, core_ids=list(range(NCORES))
        )
    except ModuleNotFoundError:
        # Tracing requested via env but the axon NTFF hook module is absent
        # in this image — rerun without tracing.
        os.environ["BASS_NEVER_TRACE"] = "1"
        res = bass_utils.run_bass_kernel_spmd(
            nc, in_maps, core_ids=list(range(NCORES))
        )

    out = np.zeros((T, D), np.float32)
    for e in range(E):
        ix = idx_lists[e]
        y = res.results[e]["y"]  # [D, C]
        out[ix] += y[:, :len(ix)].T
    return out, res


def kernel(**inputs):
    out, _ = kernel_with_results(**inputs)
    return out


# revision 10
# speedup vs baseline: 1.4646x; 1.0022x over previous
# MoE top-2 routing kernel for 8 Trainium2 NeuronCores (expert-parallel).
#
# Problem (hardcoded shapes): T=2048 tokens, D=2048 model dim, F=4096 ffn dim,
# E=8 experts, top-2 routing with renormalized softmax weights.
#
# Sharding: one expert per core. The host does the router (fp32 logits ->
# top-2 selection + renormalized softmax weights; selection is numerically
# unambiguous: min 2nd-vs-3rd prob gap ~9e-5, orders of magnitude above fp32
# matmul noise), gathers each expert's tokens into a transposed buffer
# xb [D, C] (C = max expert load rounded up to 8), and zero-pads the tail.
# Zero columns are harmless: MLP(0) = 0 and the host ignores pad columns.
#
# Device structure (all matmuls weights-STATIONARY, tokens moving): PE cost
# scales with the actual token count C (536 here) instead of 128-padded
# token tiles (5*128=640 in the v1 layout), and gate/up naturally produce
# [f, t] layout so the down matmul needs no PE transposes at all.
#   gate/up: pg[f128, C] = sum_d wg[d,f128].T @ x[d, C]   (per 128-f tile)
#   h[f, t] = silu(g) * u  (scalar+vector, bf16)
#   down:    y[d128, C] += sum_f wd[f,d128].T @ h[f, C]   (PSUM per 8-f group,
#            accumulated into SBUF ya, router-weight scaled, DMA'd out [D, C])
# Moving-dim chunks of ~C/2 <= 512 fp32 PSUM bank columns. bf16 matmuls
# stream ~1 col/cycle at the 2.0 GHz PE clock; LDWEIGHTS pipelines under the
# matmuls via the PE reorder window (trace: 139 ns per 272-col matmul, zero
# scheduling stalls).
#
# DMA: each DMA instruction costs ~700ns of issue on its queue engine and
# carries ~128 descriptors in parallel across 16 HW engines, so small tiles
# cap aggregate bandwidth (~175 GB/s at 128KB/instr). Weights load as ONE
# instruction per 512-f-chunk ([128, 16*512] rearranged), x as 4, and the
# wu/wd/wvb loads are emitted after the first gate matmuls so the startup
# critical path is just x + the first gate weight chunk.

import os
import numpy as np
import ml_dtypes

_BF16NP = ml_dtypes.bfloat16

import concourse.bass as bass
import concourse.bacc as bacc
import concourse.mybir as mybir
import concourse.tile as tile
from concourse import bass_utils

FP32 = mybir.dt.float32
BF16 = mybir.dt.bfloat16
AX = mybir.AxisListType
ALU = mybir.AluOpType
ACTF = mybir.ActivationFunctionType

T, D, F, E = 2048, 2048, 4096, 8
NCORES = 8
ND = D // 128    # 16 d-tiles (contraction for gate/up; output tiles for down)
NF = F // 128    # 32 f-tiles
G = 8            # f-tiles per down-accumulation group
NG = NF // G     # 4 groups


def _chunks8(C):
    """Split C token columns into PSUM-bank chunks (<=512 fp32, mult of 8)."""
    nch = (C + 511) // 512
    out, rem, c0 = [], C, 0
    for i in range(nch):
        cn = -(-(rem // (nch - i)) // 8) * 8
        cn = min(cn, rem)
        out.append((c0, cn))
        c0 += cn
        rem -= cn
    return out


def build_program(C):
    chunks = _chunks8(C)
    nc = bacc.Bacc(
        "TRN2",
        target_bir_lowering=False,
        debug=False,
        enable_asserts=False,
        num_devices=NCORES,
    )
    xb_d = nc.dram_tensor("xb", [D, C], BF16, kind="ExternalInput").ap()
    wvb_d = nc.dram_tensor("wvb", [128, C], FP32, kind="ExternalInput").ap()
    wg_d = nc.dram_tensor("wg", [D, F], BF16, kind="ExternalInput").ap()
    wu_d = nc.dram_tensor("wu", [D, F], BF16, kind="ExternalInput").ap()
    wd_d = nc.dram_tensor("wd", [F, D], BF16, kind="ExternalInput").ap()
    y_d = nc.dram_tensor("y", [D, C], FP32, kind="ExternalOutput").ap()

    with tile.TileContext(nc) as tc:
        with (
            tc.tile_pool(name="const", bufs=1) as const_pool,
            tc.tile_pool(name="x", bufs=1) as x_pool,
            tc.tile_pool(name="ya", bufs=1) as ya_pool,
            tc.tile_pool(name="h", bufs=2) as h_pool,
            tc.tile_pool(name="wgu", bufs=4) as wgu_pool,
            tc.tile_pool(name="wdp", bufs=2) as wd_pool,
            tc.tile_pool(name="tmp", bufs=2) as tmp_pool,
            tc.tile_pool(name="ps", bufs=6, space="PSUM") as ps_pool,
            tc.tile_pool(name="psy", bufs=2, space="PSUM") as psy_pool,
        ):
            # ---- PE warm-up: dummy matmuls on memset data while the first
            # DMAs stream in, so the HAM clock-gate reaches 8/8 (full rate)
            # before the real matmuls start. PE is otherwise idle here. ----
            warm = const_pool.tile([128, 640], BF16, tag="warm", name="warm")
            nc.vector.memset(warm[:], 0.0)
            for _ in range(10):
                pw = ps_pool.tile([128, 512], FP32, tag="ps", name="ps")
                nc.tensor.matmul(pw[:], warm[:, :128], warm[:, 128:640],
                                 start=True, stop=True)

            # ---- startup DMAs: weights on the Sync queue, x on the GpSimd
            # queue — two independent in-order queues (~350 GB/s each) so the
            # gate weights and x stream in parallel. fc0's wg/wu are split in
            # halves so the first matmuls can start after ~1MB. ----
            wg_sb = {}   # fc -> [128, 16*512] tile (all 16 d-tiles merged)
            wu_sb = {}
            wd_sb = {}   # ft -> [128, 2048] tile

            def load_w0(dram, name):
                wt = wgu_pool.tile([128, ND * 512], BF16, tag="w", name=name)
                for half in range(2):
                    nc.sync.dma_start(
                        wt[:, half * 8 * 512:(half + 1) * 8 * 512]
                        .rearrange("p (n f) -> p n f", n=8),
                        dram[half * 1024:(half + 1) * 1024, :512]
                        .rearrange("(n p) f -> p n f", p=128),
                    )
                return wt

            wg_sb[0] = load_w0(wg_d, "wgt")
            wu_sb[0] = load_w0(wu_d, "wut")

            xt4 = [x_pool.tile([128, 4 * C], BF16, tag=f"xt{q}", name=f"xt{q}")
                   for q in range(4)]
            for q in range(4):
                nc.gpsimd.dma_start(
                    xt4[q][:].rearrange("p (n c) -> p n c", n=4),
                    xb_d[q * 512:(q + 1) * 512, :]
                    .rearrange("(n p) c -> p n c", p=128),
                )

            def xsl(d, c0, cn):
                q, r = divmod(d, 4)
                return xt4[q][:, r * C + c0:r * C + c0 + cn]

            wvb = const_pool.tile([128, C], FP32, tag="wvb", name="wvb")
            ya = [ya_pool.tile([128, C], FP32, tag=f"ya{dt}", name=f"ya{dt}")
                  for dt in range(ND)]

            def load_w(dram, fc, name):
                wt = wgu_pool.tile([128, ND * 512], BF16, tag="w", name=name)
                nc.sync.dma_start(
                    wt[:].rearrange("p (n f) -> p n f", n=ND),
                    dram[:, fc * 512:(fc + 1) * 512]
                    .rearrange("(n p) f -> p n f", p=128),
                )
                return wt

            def emit_down(gprev, hprev, j):
                """Down-matmul (d-tiles 2j, 2j+1) for f-group gprev."""
                f0 = gprev * G
                for dt in (2 * j, 2 * j + 1):
                    for (c0, cn) in chunks:
                        py = psy_pool.tile([128, max(cn for _, cn in chunks)],
                                           FP32, tag="py", name="py")
                        for k in range(G):
                            nc.tensor.matmul(
                                py[:, :cn],
                                wd_sb[f0 + k][:, dt * 128:(dt + 1) * 128],
                                hprev[k][:, c0:c0 + cn],
                                start=(k == 0), stop=(k == G - 1),
                            )
                        yslc = ya[dt][:, c0:c0 + cn]
                        if gprev == 0:
                            nc.scalar.copy(yslc, py[:, :cn])
                        else:
                            nc.vector.tensor_tensor(yslc, yslc, py[:, :cn],
                                                    op=ALU.add)
                    if gprev == NG - 1:
                        nc.vector.tensor_mul(ya[dt][:], ya[dt][:], wvb[:])
                        nc.gpsimd.dma_start(
                            y_d[dt * 128:(dt + 1) * 128, :], ya[dt][:])

            hprev = None
            for g in range(NG):
                hcur = []
                for j in range(G):
                    ft = g * G + j
                    fc, fo = divmod(ft, 4)
                    if fo == 0 and fc > 0:
                        wg_sb[fc] = load_w(wg_d, fc, "wgt")
                    # gate -> pg
                    pg = [ps_pool.tile([128, cn], FP32, tag="ps", name="ps")
                          for (c0, cn) in chunks]
                    for d in range(ND):
                        wsl = wg_sb[fc][:, d * 512 + fo * 128:
                                        d * 512 + (fo + 1) * 128]
                        for ci, (c0, cn) in enumerate(chunks):
                            nc.tensor.matmul(
                                pg[ci][:], wsl, xsl(d, c0, cn),
                                start=(d == 0), stop=(d == ND - 1),
                            )
                    # up-weights / wvb DMAs go behind the first gate matmuls
                    if fo == 0 and fc > 0:
                        wu_sb[fc] = load_w(wu_d, fc, "wut")
                    if g == NG - 1 and j == 0:
                        nc.gpsimd.dma_start(wvb[:], wvb_d[:])
                    st = tmp_pool.tile([128, C], FP32, tag="st", name="st")
                    for ci, (c0, cn) in enumerate(chunks):
                        nc.scalar.activation(st[:, c0:c0 + cn], pg[ci][:],
                                             ACTF.Silu)
                    # up -> pu
                    pu = [ps_pool.tile([128, cn], FP32, tag="ps", name="ps")
                          for (c0, cn) in chunks]
                    for d in range(ND):
                        wsl = wu_sb[fc][:, d * 512 + fo * 128:
                                        d * 512 + (fo + 1) * 128]
                        for ci, (c0, cn) in enumerate(chunks):
                            nc.tensor.matmul(
                                pu[ci][:], wsl, xsl(d, c0, cn),
                                start=(d == 0), stop=(d == ND - 1),
                            )
                    ht = h_pool.tile([128, C], BF16, tag=f"h{j}", name=f"h{j}")
                    for ci, (c0, cn) in enumerate(chunks):
                        nc.vector.tensor_mul(ht[:, c0:c0 + cn],
                                             st[:, c0:c0 + cn], pu[ci][:])
                    hcur.append(ht)
                    # down-weights for this f-tile (used next group)
                    wdt = wd_pool.tile([128, D], BF16, tag=f"wd{j}", name="wdt")
                    nc.sync.dma_start(wdt[:], wd_d[ft * 128:(ft + 1) * 128, :])
                    wd_sb[ft] = wdt
                    if hprev is not None:
                        emit_down(g - 1, hprev, j)
                hprev = hcur
            for j in range(G):
                emit_down(NG - 1, hprev, j)

    nc.compile()
    return nc


_PROGRAM_CACHE = {}


def _get_program(C):
    if C not in _PROGRAM_CACHE:
        _PROGRAM_CACHE[C] = build_program(C)
    return _PROGRAM_CACHE[C]


def _route_host(x_TD, router_w):
    """Host router: top-2 ids + renormalized softmax weights per token."""
    logits = (x_TD @ router_w).astype(np.float64)  # [T, E]
    logits -= logits.max(axis=1, keepdims=True)
    probs = np.exp(logits)
    probs /= probs.sum(axis=1, keepdims=True)
    order = np.argsort(-probs, axis=1, kind="stable")
    top2 = order[:, :2]
    w12 = np.take_along_axis(probs, top2, axis=1)
    w12 /= w12.sum(axis=1, keepdims=True)
    return top2, w12.astype(np.float32)


def kernel_with_results(x_TD, router_w, w_gate, w_up, w_down):
    x_TD = np.ascontiguousarray(x_TD, np.float32)
    router_w = np.ascontiguousarray(router_w, np.float32)

    top2, w12 = _route_host(x_TD, router_w)
    idx_lists, wt_lists = [], []
    for e in range(E):
        sel = top2 == e  # [T, 2]
        any_sel = sel.any(axis=1)
        ix = np.where(any_sel)[0]
        idx_lists.append(ix)
        wt_lists.append(w12[any_sel][sel[ix]])
    max_cnt = max(len(ix) for ix in idx_lists)
    C = max(64, -(-max_cnt // 8) * 8)

    nc = _get_program(C)

    xT = np.ascontiguousarray(x_TD.T).astype(_BF16NP)  # [D, T] bf16
    in_maps = []
    for e in range(E):
        ix = idx_lists[e]
        xb = np.zeros((D, C), _BF16NP)
        xb[:, :len(ix)] = xT[:, ix]
        wvb = np.zeros((1, C), np.float32)
        wvb[0, :len(ix)] = wt_lists[e]
        in_maps.append({
            "xb": xb,
            "wvb": np.ascontiguousarray(np.broadcast_to(wvb, (128, C))),
            "wg": w_gate[e].astype(_BF16NP),
            "wu": w_up[e].astype(_BF16NP),
            "wd": w_down[e].astype(_BF16NP),
        })

    try:
        res = bass_utils.run_bass_kernel_spmd(
            nc, in_maps, core_ids=list(range(NCORES))
        )
    except ModuleNotFoundError:
        # Tracing requested via env but the axon NTFF hook module is absent
        # in this image — rerun without tracing.
        os.environ["BASS_NEVER_TRACE"] = "1"
        res = bass_utils.run_bass_kernel_spmd(
            nc, in_maps, core_ids=list(range(NCORES))
        )

    out = np.zeros((T, D), np.float32)
    for e in range(E):
        ix = idx_lists[e]
        y = res.results[e]["y"]  # [D, C]
        out[ix] += y[:, :len(ix)].T
    return out, res


def kernel(**inputs):
    out, _ = kernel_with_results(**inputs)
    return out


# revision 13
# speedup vs baseline: 1.4744x; 1.0067x over previous
# MoE top-2 routing kernel for 8 Trainium2 NeuronCores (expert-parallel).
#
# Problem (hardcoded shapes): T=2048 tokens, D=2048 model dim, F=4096 ffn dim,
# E=8 experts, top-2 routing with renormalized softmax weights.
#
# Sharding: one expert per core. The host does the router (fp32 logits ->
# top-2 selection + renormalized softmax weights; selection is numerically
# unambiguous: min 2nd-vs-3rd prob gap ~9e-5, orders of magnitude above fp32
# matmul noise), gathers each expert's tokens into a transposed buffer
# xb [D, C] (C = max expert load rounded up to 8), and zero-pads the tail.
# Zero columns are harmless: MLP(0) = 0 and the host ignores pad columns.
#
# Device structure (all matmuls weights-STATIONARY, tokens moving): PE cost
# scales with the actual token count C (536 here) instead of 128-padded
# token tiles (5*128=640 in the v1 layout), and gate/up naturally produce
# [f, t] layout so the down matmul needs no PE transposes at all.
#   gate/up: pg[f128, C] = sum_d wg[d,f128].T @ x[d, C]   (per 128-f tile)
#   h[f, t] = silu(g) * u  (scalar+vector, bf16)
#   down:    y[d128, C] += sum_f wd[f,d128].T @ h[f, C]   (PSUM per 8-f group,
#            accumulated into SBUF ya, router-weight scaled, DMA'd out [D, C])
# Moving-dim chunks of ~C/2 <= 512 fp32 PSUM bank columns. bf16 matmuls
# stream ~1 col/cycle at the 2.0 GHz PE clock; LDWEIGHTS pipelines under the
# matmuls via the PE reorder window (trace: 139 ns per 272-col matmul, zero
# scheduling stalls).
#
# DMA: each DMA instruction costs ~700ns of issue on its queue engine and
# carries ~128 descriptors in parallel across 16 HW engines, so small tiles
# cap aggregate bandwidth (~175 GB/s at 128KB/instr). Weights load as ONE
# instruction per 512-f-chunk ([128, 16*512] rearranged), x as 4, and the
# wu/wd/wvb loads are emitted after the first gate matmuls so the startup
# critical path is just x + the first gate weight chunk.

import os
import numpy as np
import ml_dtypes

_BF16NP = ml_dtypes.bfloat16

import concourse.bass as bass
import concourse.bacc as bacc
import concourse.mybir as mybir
import concourse.tile as tile
from concourse import bass_utils

FP32 = mybir.dt.float32
BF16 = mybir.dt.bfloat16
AX = mybir.AxisListType
ALU = mybir.AluOpType
ACTF = mybir.ActivationFunctionType

T, D, F, E = 2048, 2048, 4096, 8
NCORES = 8
ND = D // 128    # 16 d-tiles (contraction for gate/up; output tiles for down)
NF = F // 128    # 32 f-tiles
G = 8            # f-tiles per down-accumulation group
NG = NF // G     # 4 groups


def _chunks8(C):
    """Split C token columns into PSUM-bank chunks (<=512 fp32, mult of 8)."""
    nch = (C + 511) // 512
    out, rem, c0 = [], C, 0
    for i in range(nch):
        cn = -(-(rem // (nch - i)) // 8) * 8
        cn = min(cn, rem)
        out.append((c0, cn))
        c0 += cn
        rem -= cn
    return out


def build_program(C):
    chunks = _chunks8(C)
    nc = bacc.Bacc(
        "TRN2",
        target_bir_lowering=False,
        debug=False,
        enable_asserts=False,
        num_devices=NCORES,
    )
    xb_d = nc.dram_tensor("xb", [D, C], BF16, kind="ExternalInput").ap()
    wvb_d = nc.dram_tensor("wvb", [128, C], FP32, kind="ExternalInput").ap()
    wg_d = nc.dram_tensor("wg", [D, F], BF16, kind="ExternalInput").ap()
    wu_d = nc.dram_tensor("wu", [D, F], BF16, kind="ExternalInput").ap()
    wd_d = nc.dram_tensor("wd", [F, D], BF16, kind="ExternalInput").ap()
    y_d = nc.dram_tensor("y", [D, C], FP32, kind="ExternalOutput").ap()

    with tile.TileContext(nc) as tc:
        with (
            tc.tile_pool(name="const", bufs=1) as const_pool,
            tc.tile_pool(name="x", bufs=1) as x_pool,
            tc.tile_pool(name="ya", bufs=1) as ya_pool,
            tc.tile_pool(name="h", bufs=2) as h_pool,
            tc.tile_pool(name="wgu", bufs=4) as wgu_pool,
            tc.tile_pool(name="wdp", bufs=2) as wd_pool,
            tc.tile_pool(name="tmp", bufs=2) as tmp_pool,
            tc.tile_pool(name="ps", bufs=6, space="PSUM") as ps_pool,
            tc.tile_pool(name="psy", bufs=2, space="PSUM") as psy_pool,
        ):
            # ---- PE warm-up: dummy matmuls on memset data while the first
            # DMAs stream in, so the HAM clock-gate reaches 8/8 (full rate)
            # before the real matmuls start. PE is otherwise idle here. ----
            warm = const_pool.tile([128, 640], BF16, tag="warm", name="warm")
            nc.vector.memset(warm[:], 0.0)
            for _ in range(13):
                pw = ps_pool.tile([128, 512], FP32, tag="ps", name="ps")
                nc.tensor.matmul(pw[:], warm[:, :128], warm[:, 128:640],
                                 start=True, stop=True)

            # ---- startup DMAs: weights on the Sync queue, x on the GpSimd
            # queue — two independent in-order queues (~350 GB/s each) so the
            # gate weights and x stream in parallel. fc0's wg/wu are split in
            # halves so the first matmuls can start after ~1MB. ----
            wg_sb = {}   # fc -> [128, 16*512] tile (all 16 d-tiles merged)
            wu_sb = {}
            wd_sb = {}   # ft -> [128, 2048] tile

            def load_w0(dram, name):
                wt = wgu_pool.tile([128, ND * 512], BF16, tag="w", name=name)
                for half in range(2):
                    nc.sync.dma_start(
                        wt[:, half * 8 * 512:(half + 1) * 8 * 512]
                        .rearrange("p (n f) -> p n f", n=8),
                        dram[half * 1024:(half + 1) * 1024, :512]
                        .rearrange("(n p) f -> p n f", p=128),
                    )
                return wt

            wg_sb[0] = load_w0(wg_d, "wgt")
            wu_sb[0] = load_w0(wu_d, "wut")

            xt4 = [x_pool.tile([128, 4 * C], BF16, tag=f"xt{q}", name=f"xt{q}")
                   for q in range(4)]
            for q in range(4):
                nc.gpsimd.dma_start(
                    xt4[q][:].rearrange("p (n c) -> p n c", n=4),
                    xb_d[q * 512:(q + 1) * 512, :]
                    .rearrange("(n p) c -> p n c", p=128),
                )

            def xsl(d, c0, cn):
                q, r = divmod(d, 4)
                return xt4[q][:, r * C + c0:r * C + c0 + cn]

            wvb = const_pool.tile([128, C], FP32, tag="wvb", name="wvb")
            ya = [ya_pool.tile([128, C], FP32, tag=f"ya{dt}", name=f"ya{dt}")
                  for dt in range(ND)]

            def load_w(dram, fc, name):
                wt = wgu_pool.tile([128, ND * 512], BF16, tag="w", name=name)
                nc.sync.dma_start(
                    wt[:].rearrange("p (n f) -> p n f", n=ND),
                    dram[:, fc * 512:(fc + 1) * 512]
                    .rearrange("(n p) f -> p n f", p=128),
                )
                return wt

            def emit_down(gprev, hprev, j):
                """Down-matmul (d-tiles 2j, 2j+1) for f-group gprev."""
                f0 = gprev * G
                for dt in (2 * j, 2 * j + 1):
                    for (c0, cn) in chunks:
                        py = psy_pool.tile([128, max(cn for _, cn in chunks)],
                                           FP32, tag="py", name="py")
                        for k in range(G):
                            nc.tensor.matmul(
                                py[:, :cn],
                                wd_sb[f0 + k][:, dt * 128:(dt + 1) * 128],
                                hprev[k][:, c0:c0 + cn],
                                start=(k == 0), stop=(k == G - 1),
                            )
                        yslc = ya[dt][:, c0:c0 + cn]
                        if gprev == 0:
                            nc.scalar.copy(yslc, py[:, :cn])
                        else:
                            nc.vector.tensor_tensor(yslc, yslc, py[:, :cn],
                                                    op=ALU.add)
                    if gprev == NG - 1:
                        nc.vector.tensor_mul(ya[dt][:], ya[dt][:], wvb[:])
                        nc.gpsimd.dma_start(
                            y_d[dt * 128:(dt + 1) * 128, :], ya[dt][:])

            def emit_gate(fc, fo):
                pg = [ps_pool.tile([128, cn], FP32, tag="ps", name="ps")
                      for (c0, cn) in chunks]
                for d in range(ND):
                    wsl = wg_sb[fc][:, d * 512 + fo * 128:
                                    d * 512 + (fo + 1) * 128]
                    for ci, (c0, cn) in enumerate(chunks):
                        nc.tensor.matmul(
                            pg[ci][:], wsl, xsl(d, c0, cn),
                            start=(d == 0), stop=(d == ND - 1),
                        )
                st = tmp_pool.tile([128, C], FP32, tag="st", name="st",
                                   bufs=4)
                for ci, (c0, cn) in enumerate(chunks):
                    nc.scalar.activation(st[:, c0:c0 + cn], pg[ci][:],
                                         ACTF.Silu)
                return st

            def emit_up(fc, fo, st, j):
                pu = [ps_pool.tile([128, cn], FP32, tag="ps", name="ps")
                      for (c0, cn) in chunks]
                for d in range(ND):
                    wsl = wu_sb[fc][:, d * 512 + fo * 128:
                                    d * 512 + (fo + 1) * 128]
                    for ci, (c0, cn) in enumerate(chunks):
                        nc.tensor.matmul(
                            pu[ci][:], wsl, xsl(d, c0, cn),
                            start=(d == 0), stop=(d == ND - 1),
                        )
                ht = h_pool.tile([128, C], BF16, tag=f"h{j}", name=f"h{j}")
                for ci, (c0, cn) in enumerate(chunks):
                    nc.vector.tensor_mul(ht[:, c0:c0 + cn],
                                         st[:, c0:c0 + cn], pu[ci][:])
                ft = (fc * 4 + fo)
                wdt = wd_pool.tile([128, D], BF16, tag=f"wd{j}", name="wdt")
                nc.sync.dma_start(wdt[:], wd_d[ft * 128:(ft + 1) * 128, :])
                wd_sb[ft] = wdt
                return ht

            hprev = None
            for g in range(NG):
                hcur = []
                for j in range(G):
                    ft = g * G + j
                    fc, fo = divmod(ft, 4)
                    if fc == 0:
                        # fc0: run all 4 gate f-tiles before the first up so
                        # the PE has work while the up-weights stream in.
                        if j == 0:
                            sts0 = [emit_gate(0, ff) for ff in range(4)]
                        ht = emit_up(0, fo, sts0[fo], j)
                    else:
                        if fo == 0:
                            wg_sb[fc] = load_w(wg_d, fc, "wgt")
                        st = emit_gate(fc, fo)
                        # up-weights DMA goes behind this f-tile's gate MMs
                        if fo == 0:
                            wu_sb[fc] = load_w(wu_d, fc, "wut")
                        ht = emit_up(fc, fo, st, j)
                    if g == NG - 1 and j == 0:
                        nc.gpsimd.dma_start(wvb[:], wvb_d[:])
                    hcur.append(ht)
                    if hprev is not None:
                        emit_down(g - 1, hprev, j)
                hprev = hcur
            for j in range(G):
                emit_down(NG - 1, hprev, j)

    nc.compile()
    return nc


_PROGRAM_CACHE = {}


def _get_program(C):
    if C not in _PROGRAM_CACHE:
        _PROGRAM_CACHE[C] = build_program(C)
    return _PROGRAM_CACHE[C]


def _route_host(x_TD, router_w):
    """Host router: top-2 ids + renormalized softmax weights per token."""
    logits = (x_TD @ router_w).astype(np.float64)  # [T, E]
    logits -= logits.max(axis=1, keepdims=True)
    probs = np.exp(logits)
    probs /= probs.sum(axis=1, keepdims=True)
    order = np.argsort(-probs, axis=1, kind="stable")
    top2 = order[:, :2]
    w12 = np.take_along_axis(probs, top2, axis=1)
    w12 /= w12.sum(axis=1, keepdims=True)
    return top2, w12.astype(np.float32)


def kernel_with_results(x_TD, router_w, w_gate, w_up, w_down):
    x_TD = np.ascontiguousarray(x_TD, np.float32)
    router_w = np.ascontiguousarray(router_w, np.float32)

    top2, w12 = _route_host(x_TD, router_w)
    idx_lists, wt_lists = [], []
    for e in range(E):
        sel = top2 == e  # [T, 2]
        any_sel = sel.any(axis=1)
        ix = np.where(any_sel)[0]
        idx_lists.append(ix)
        wt_lists.append(w12[any_sel][sel[ix]])
    max_cnt = max(len(ix) for ix in idx_lists)
    C = max(64, -(-max_cnt // 8) * 8)

    nc = _get_program(C)

    xT = np.ascontiguousarray(x_TD.T).astype(_BF16NP)  # [D, T] bf16
    in_maps = []
    for e in range(E):
        ix = idx_lists[e]
        xb = np.zeros((D, C), _BF16NP)
        xb[:, :len(ix)] = xT[:, ix]
        wvb = np.zeros((1, C), np.float32)
        wvb[0, :len(ix)] = wt_lists[e]
        in_maps.append({
            "xb": xb,
            "wvb": np.ascontiguousarray(np.broadcast_to(wvb, (128, C))),
            "wg": w_gate[e].astype(_BF16NP),
            "wu": w_up[e].astype(_BF16NP),
            "wd": w_down[e].astype(_BF16NP),
        })

    try:
        res = bass_utils.run_bass_kernel_spmd(
            nc, in_maps, core_ids=list(range(NCORES))
        )
    except ModuleNotFoundError:
        # Tracing requested via env but the axon NTFF hook module is absent
        # in this image — rerun without tracing.
        os.environ["BASS_NEVER_TRACE"] = "1"
        res = bass_utils.run_bass_kernel_spmd(
            nc, in_maps, core_ids=list(range(NCORES))
        )

    out = np.zeros((T, D), np.float32)
    for e in range(E):
        ix = idx_lists[e]
        y = res.results[e]["y"]  # [D, C]
        out[ix] += y[:, :len(ix)].T
    return out, res


def kernel(**inputs):
    out, _ = kernel_with_results(**inputs)
    return out
